# revision 85
# baseline (speedup 1.0000x reference)
"""Trainium2 Bass kernel for nn_Net_7241314861573 (forward-forward net predict).

Strategy: data-parallel over batch. 8 cores x 16 samples; each core handles
160 rows (r = s*10 + l over its 16 samples x 10 labels).

Precision: fp16 x fp16 matmuls (1 cycle/row on the PE, 4x the fp32 rate;
half the weight DMA). Weight rounding error is common-mode across the 10
label-rows of a sample (same W, nearly identical states), so it cancels in
the label-argmax; the x side is the differential part and fp16's 11 bits
are enough (bf16's 8 are not). Empirically 0/128 argmax flips, same as
fp32. Operands are pre-scaled (W x 2^10, x-side x 2^12) so no value comes
near the fp16 subnormal range; the 2^-22 descale is folded into the drain
activation's scale argument. The blur/threshold path (mask = blur > 0.5)
stays in fp32: blurred values sit near 0.5 and fp16 would flip mask bits.

Math reductions baked in:
  - t0 states are zero => layer(0,W,b) = relu(b): constant terms c1/c2/c3t0.
  - pre-input of layer1 is always h  => hp07 = 0.7*layer(h,Wp1,bp1) computed
    once and reused at t0/t1/t2.
  - Wq3 has zero-width input => 0.7*relu(bq3) constant (cq3).
  - 5x box-blur == matmul with G = kron(A,A).T, A = T^5 tridiagonal(1/3);
    fp32 G-matmul mask is bit-exact vs the reference conv. G is block-banded
    (|kt-mo| <= 2); the 20 exactly-zero 128-blocks are skipped.
  - the goodness terms for n1/n2 at t==2 are the row-sumsq that norm()
    computes anyway — captured from its PSUM row for free.

Schedule/bandwidth notes (device time ~0.40ms vs 1.24ms for the fp32
version of the same schedule; 16 streamed 8MB fp16 weight fetches put the
run right at the ~350GB/s per-core DMA roofline, PE ~82% busy):
  - Wp2 (3 uses) is SBUF-resident, fetched once during the h-phase DMA
    trough; its DMAs are interleaved BEHIND urgent transfers because
    packets stripe FIFO across all 16 DMA engines.
  - norm() for s1/s2/s3 is hoisted into the after_g0 slot of the big
    matmul that overlaps it, so no matmul ever waits on a norm chain.
  - wp1 and t0-Wp2/Ws1 consume UN-normalized fp16 casts; the
    XSCALE/(||row||+EPS) broadcast is applied inside their drains
    (tensor_tensor takes one PSUM operand, so bc is copied to SBUF).

Layouts (T-layout): state[p, kt, r] = state_row[r, kt*128 + p].
Weights prepacked host-side into per-group contiguous lhsT blocks.
"""

import numpy as np

L, B, IN, H = 10, 128, 784, 2048
EPS = 1e-4
WSCALE = 1024.0            # 2^10: weight scale before fp16 cast
XSCALE = 4096.0            # 2^12: x-side scale folded into norm()
DESCALE = 1.0 / (WSCALE * XSCALE)   # folded into drain activation scales
NC_ = 8            # cores
SPC = B // NC_     # 16 samples per core
R = SPC * L        # 160 rows per core
KT = H // 128      # 16 k-chunks for H
KTH = 7            # k-chunks for padded input 896
INP = KTH * 128    # 896
NG = 4             # weight DMA groups (4 m-chunks of 128 = 512 cols each)

WNAMES = ["Ws1", "Wq1", "Wp2", "Ws2", "Wq2", "Wp3", "Ws3"]
WIDX = {n: i for i, n in enumerate(WNAMES)}


# ---------------------------------------------------------------- host prep

def _blur_matrix():
    Td = np.zeros((28, 28))
    for i in range(28):
        for j in (i - 1, i, i + 1):
            if 0 <= j < 28:
                Td[i, j] = 1.0 / 3.0
    A = np.linalg.matrix_power(Td, 5)
    G = np.kron(A, A).T.astype(np.float32)
    Gp = np.zeros((INP, INP), np.float32)
    Gp[:IN, :IN] = G
    # A has bandwidth 5, so G is block-banded: 128-blocks with |kt-mo| > 2
    # are exactly zero (|r-c| <= 5*28+5 = 145 < 256). The device skips them.
    for kt in range(KTH):
        for mo in range(KTH):
            if abs(kt - mo) > 2:
                assert not Gp[kt * 128:(kt + 1) * 128,
                              mo * 128:(mo + 1) * 128].any()
    return np.ascontiguousarray(Gp.reshape(KTH, 128, INP).transpose(1, 0, 2))


def _pack_w(WT_pad, ktn):
    # WT_pad: [ktn*128, 2048] -> [NG, 128, ktn, 512] fp16 contiguous per group
    # (2MB groups beat 4MB halves: finer whole-tile DMA deps pipeline better)
    a = (WT_pad * np.float32(WSCALE)).astype(np.float16)
    a = a.reshape(ktn, 128, NG, 512).transpose(2, 1, 0, 3)
    return [np.ascontiguousarray(a[g]) for g in range(NG)]


def _col(v):
    # [2048] -> [128, 16] with col[p, m] = v[m*128 + p]
    return v.reshape(KT, 128).T


def prep_shared(inputs):
    f32 = np.float32
    sh = {}
    sh["gmat"] = _blur_matrix()

    for n in WNAMES:
        W = np.asarray(inputs[n], f32)
        for g, arr in enumerate(_pack_w(np.ascontiguousarray(W.T), KT)):
            sh[f"w{WIDX[n]}_g{g}"] = arr

    Wp1T = np.zeros((INP, H), f32)
    Wp1T[:IN] = np.asarray(inputs["Wp1"], f32).T
    for g, arr in enumerate(_pack_w(Wp1T, KTH)):
        sh[f"wp1_g{g}"] = arr

    b = {k: np.asarray(inputs[k], f32) for k in
         ("bp1", "bq1", "bs1", "bp2", "bq2", "bs2", "bp3", "bs3", "bq3")}
    r = {k: np.maximum(v, f32(0)) for k, v in b.items()}
    c7, c3 = f32(0.7), f32(0.3)
    cols = [
        c7 * b["bp1"], c7 * b["bq1"], c3 * b["bs1"],
        c7 * b["bp2"], c7 * b["bq2"], c3 * b["bs2"],
        c7 * b["bp3"], c3 * b["bs3"],
        c7 * r["bq1"] + c3 * r["bs1"],
        c7 * r["bq2"] + c3 * r["bs2"],
        c7 * r["bq3"] + c3 * r["bs3"],
        c7 * r["bq3"],
    ]
    bt = np.zeros((128, KT, 12), f32)
    for ci, v in enumerate(cols):
        bt[:, :, ci] = _col(v.astype(f32))
    sh["bt"] = bt
    return sh


def _tlay(rows):
    # rows: [R, INP] -> [128, KTH, R]
    return np.ascontiguousarray(rows.reshape(R, KTH, 128).transpose(2, 1, 0))


def prep_core(inputs, c):
    f32 = np.float32
    x = np.asarray(inputs["x"], f32)            # [B, IN]
    mn = np.asarray(inputs["mask_noise"], f32)  # [L, B, 28, 28]
    mix = np.asarray(inputs["mix_idx"])         # [L, B] int
    xmax = x.max()

    gb = np.arange(c * SPC, (c + 1) * SPC)      # global sample ids
    # row r = s*10 + l
    ls = np.tile(np.arange(L), SPC)             # label per row
    ss = np.repeat(gb, L)                       # global sample per row

    mnr = np.zeros((R, INP), f32)
    mnr[:, :IN] = mn[ls, ss].reshape(R, IN)

    lab = np.zeros((R, L), f32)
    lab[np.arange(R), ls] = xmax

    xtr = np.zeros((R, INP), f32)
    xtr[:, :IN] = x[ss]
    xtr[:, :L] = lab

    xmixr = np.zeros((R, INP), f32)
    xmixr[:, :IN] = x[mix[ls, ss]]
    xmixr[:, :L] = lab

    return {"mnt": _tlay(mnr), "xt": _tlay(xtr), "xmixt": _tlay(xmixr)}


# ---------------------------------------------------------------- bass program

def build_program(mode="full"):
    import concourse.bacc as bacc
    import concourse.mybir as mybir
    import concourse.tile as tile

    fp32 = mybir.dt.float32
    fp16 = mybir.dt.float16
    AF = mybir.ActivationFunctionType
    nc = bacc.Bacc()

    dr = {}
    for i in range(len(WNAMES)):
        for g in range(NG):
            dr[f"w{i}_g{g}"] = nc.dram_tensor(
                f"w{i}_g{g}", [128, KT, 512], fp16, kind="ExternalInput")
    for g in range(NG):
        dr[f"wp1_g{g}"] = nc.dram_tensor(
            f"wp1_g{g}", [128, KTH, 512], fp16, kind="ExternalInput")
    dr["gmat"] = nc.dram_tensor("gmat", [128, KTH, INP], fp32, kind="ExternalInput")
    dr["bt"] = nc.dram_tensor("bt", [128, KT, 12], fp32, kind="ExternalInput")
    for n in ("mnt", "xt", "xmixt"):
        dr[n] = nc.dram_tensor(n, [128, KTH, R], fp32, kind="ExternalInput")
    out_d = nc.dram_tensor("out", [SPC, 1], fp32, kind="ExternalOutput")

    if mode == "dma":
        # stream the exact same weight traffic as "full", trivial consumer
        seq = [2, 5] + [1, 0, 4, 3, 2, 6, 5] * 2
        with tile.TileContext(nc) as tc:
            with (
                tc.tile_pool(name="persist", bufs=1) as pp,
                tc.tile_pool(name="wstream", bufs=3) as wp,
            ):
                acc = pp.tile([128, 1], fp32, tag="acc")
                nc.vector.memset(acc[:], 0.0)
                bt = pp.tile([128, KT, 12], fp32, tag="bt")
                nc.sync.dma_start(bt[:], dr["bt"][:])
                gm = pp.tile([128, KTH, INP], fp32, tag="gm")
                nc.sync.dma_start(gm[:], dr["gmat"][:])
                for n in ("mnt", "xt", "xmixt"):
                    st = wp.tile([128, KTH, R], fp32, tag="st")
                    nc.sync.dma_start(st[:], dr[n][:])
                    nc.vector.tensor_add(acc[:], acc[:], st[:, 0, 0:1])
                for g in range(NG):
                    w1 = wp.tile([128, KTH, 512], fp16, tag="w1")
                    nc.sync.dma_start(w1[:], dr[f"wp1_g{g}"][:])
                    nc.vector.tensor_copy(acc[0:1, 0:1], w1[0:1, 0, 0:1])
                for i in seq:
                    for g in range(NG):
                        wt = wp.tile([128, KT, 512], fp16, tag="w")
                        nc.sync.dma_start(wt[:], dr[f"w{i}_g{g}"][:])
                        nc.vector.tensor_copy(acc[0:1, 0:1], wt[0:1, 0, 0:1])
                nc.vector.tensor_add(acc[:], acc[:], gm[:, 0, 0:1])
                nc.vector.tensor_add(acc[:], acc[:], bt[:, 0, 0:1])
                nc.sync.dma_start(out_d[:], acc[0:SPC, 0:1])
        nc.finalize()
        return nc

    with tile.TileContext(nc) as tc:
        with (
            tc.tile_pool(name="persist", bufs=1) as pp,
            tc.tile_pool(name="tmp", bufs=3) as tp,
            tc.tile_pool(name="rw", bufs=1) as rp,
            tc.tile_pool(name="pz", bufs=5, space="PSUM") as pz,
            tc.tile_pool(name="pn", bufs=1, space="PSUM") as pn,
        ):
            # Wp2 (3 uses: t0/t1/t2) stays SBUF-resident: fetched once during
            # the h-phase DMA trough, saving 2 refetches (16MB of HBM traffic)
            wp2res = [rp.tile([128, KT, 512], fp16, tag=f"wp2r{g}",
                              name=f"wp2r{g}")
                      for g in range(NG)]
            res_list = [(wp2res[g], f"w{WIDX['Wp2']}_g{g}") for g in range(NG)]
            s1 = pp.tile([128, KT, R], fp32, tag="s1")
            s2 = pp.tile([128, KT, R], fp32, tag="s2")
            s3 = pp.tile([128, KT, R], fp32, tag="s3")
            d1 = pp.tile([128, KT, R], fp16, tag="d1")
            d2 = pp.tile([128, KT, R], fp16, tag="d2")
            d3 = pp.tile([128, KT, R], fp16, tag="d3")
            hp07 = pp.tile([128, KT, R], fp32, tag="hp07")
            sq = pp.tile([128, KT, R], fp32, tag="sq")
            bt = pp.tile([128, KT, 12], fp32, tag="bt")
            ones_col = pp.tile([128, 1], fp32, tag="ones_col")
            ones_row = pp.tile([1, 128], fp32, tag="ones_row")
            ssq = pp.tile([1, R], fp32, tag="ssq")
            inv = pp.tile([1, R], fp32, tag="inv")
            grow = pp.tile([1, R], fp32, tag="grow")
            mxrow = pp.tile([1, 8 * SPC], fp32, tag="mxrow")
            ixrow = pp.tile([1, 8 * SPC], mybir.dt.uint32, tag="ixrow")
            outf = pp.tile([1, SPC], fp32, tag="outf")

            nc.vector.memset(ones_col[:], 1.0)
            nc.vector.memset(ones_row[:], 1.0)
            nc.sync.dma_start(bt[:], dr["bt"][:])

            def make_bc(ktn=KT, skip_sq=False, src=None, g_capture=None,
                        xscale=XSCALE, to_sbuf=False):
                """bc[:, r] = xscale/(sqrt(sumsq_row) + EPS) from the squares
                in sq (or squaring src first), broadcast across partitions.

                g_capture ("init"/"add"): fold this row-sumsq into grow — the
                goodness term mean(src^2) up to the label-constant 1/H factor,
                bit-identical to a separate square/tree/rowsum pass."""
                if not skip_sq:
                    nc.scalar.activation(sq[:, :ktn, :], src[:, :ktn, :], AF.Square)
                    n = ktn
                    while n > 1:
                        half = n // 2
                        rem = n - half
                        nc.vector.tensor_add(
                            sq[:, 0:half, :], sq[:, 0:half, :], sq[:, rem:n, :])
                        n = rem
                ssp = pn.tile([128, 512], fp32, tag="ss")
                nc.tensor.matmul(ssp[0:1, :R], ones_col[:], sq[:, 0, :],
                                 start=True, stop=True)
                if g_capture == "init":
                    nc.vector.tensor_copy(grow[:], ssp[0:1, :R])
                elif g_capture == "add":
                    nc.vector.tensor_add(grow[:], grow[:], ssp[0:1, :R])
                # sqrt(sumsq)/xscale so inv = xscale/(||row|| + EPS): the
                # normalized fp16 output is pre-scaled away from subnormals
                nc.scalar.activation(ssq[:], ssp[0:1, :R], AF.Sqrt,
                                     scale=1.0 / (xscale * xscale))
                nc.vector.tensor_scalar_add(ssq[:], ssq[:], float(EPS) / xscale)
                nc.vector.reciprocal(inv[:], ssq[:])
                bc = pn.tile([128, 512], fp32, tag="bc", bufs=2)
                nc.tensor.matmul(bc[:, :R], ones_row[:], inv[:],
                                 start=True, stop=True)
                if to_sbuf:
                    # drains multiply bc against a PSUM matmul result, and
                    # tensor_tensor cannot take two PSUM operands
                    bcs = tp.tile([128, R], fp32, tag="bcs", bufs=2)
                    nc.vector.tensor_copy(bcs[:], bc[:, :R])
                    return bcs
                return bc

            def norm(src, dst, ktn=KT, skip_sq=False, g_capture=None):
                """dst = src * xscale/(sqrt(sumsq_row(src)) + EPS), as fp16."""
                bc = make_bc(ktn=ktn, skip_sq=skip_sq, src=src,
                             g_capture=g_capture)
                for kt in range(ktn):
                    nc.vector.tensor_mul(dst[:, kt, :], src[:, kt, :], bc[:, :R])

            # ---------------- h phase: blur mask, hybrid, norm, Wp1 ----------
            with (
                tc.tile_pool(name="hph", bufs=1) as hp,
                tc.tile_pool(name="w1p", bufs=3) as w1p,
            ):
                gm = hp.tile([128, KTH, 640], fp32, tag="gm")
                mnt = hp.tile([128, KTH, R], fp32, tag="mnt")
                xt = hp.tile([128, KTH, R], fp32, tag="xt")
                h = hp.tile([128, KTH, R], fp32, tag="h")
                dh = hp.tile([128, KTH, R], fp16, tag="dh")
                # per-kt split so the first blur matmul starts ~7x sooner
                # resident-weight DMAs fill the h-phase bandwidth trough, but
                # packets stripe FIFO across all DMA engines, so interleave
                # them BEHIND each urgent transfer (gmat/mnt first, then the
                # wp1 groups) instead of queueing 10MB ahead of everything
                def res_dma(i):
                    if mode != "pe":
                        t_, nm = res_list[i]
                        nc.sync.dma_start(t_[:], dr[nm][:])

                for kt in range(KTH):
                    nc.sync.dma_start(mnt[:, kt, :], dr["mnt"][:, kt, :])
                    c0, c1 = max(0, kt - 2) * 128, min(KTH, kt + 3) * 128
                    nc.sync.dma_start(gm[:, kt, 0:c1 - c0],
                                      dr["gmat"][:, kt, c0:c1])
                for kt in range(KTH):
                    nc.sync.dma_start(xt[:, kt, :], dr["xt"][:, kt, :])
                    nc.sync.dma_start(h[:, kt, :], dr["xmixt"][:, kt, :])
                # wp1 g0-g2 pre-issued AHEAD of the resident weights: they're
                # needed right after the blur, the residents only at t0+
                w1_pre = []
                if mode != "pe":
                    for g in range(3):
                        w1t = w1p.tile([128, KTH, 512], fp16, tag="w1",
                                       name=f"w1pre{g}")
                        nc.sync.dma_start(w1t[:], dr[f"wp1_g{g}"][:])
                        w1_pre.append(w1t)
                res_dma(0)
                res_dma(1)

                for mo in range(KTH):
                    zp = pz.tile([128, 512], fp32, tag="z")
                    zv = zp[:, :R]
                    kts = range(max(0, mo - 2), min(KTH, mo + 3))
                    for kt in kts:
                        co = (mo - max(0, kt - 2)) * 128
                        nc.tensor.matmul(
                            zv, gm[:, kt, co:co + 128],
                            mnt[:, kt, :],
                            start=(kt == kts[0]), stop=(kt == kts[-1]))
                    pred = tp.tile([128, R], mybir.dt.uint8, tag="pred")
                    nc.vector.tensor_scalar(
                        pred[:], zv, 0.5, None, mybir.AluOpType.is_gt)
                    # where blur>0.5 use own image x
                    nc.vector.copy_predicated(h[:, mo, :], pred[:], xt[:, mo, :])
                    # dh = fp16(h) UN-normalized: the Wp1 matmuls start as soon
                    # as the casts land; the 1/(||h||+EPS) chain runs under
                    # them and is only needed at drain time (as a bc multiply)
                    nc.vector.tensor_copy(dh[:, mo, :], h[:, mo, :])

                # sumsq for norm(h) as slab ops — off the wp1 critical path
                # (bch is only needed at drain time), and far fewer engine ops
                # than per-chunk squares would queue up
                nc.scalar.activation(sq[:, 0:KTH, :], h[:, 0:KTH, :], AF.Square)
                nc.vector.tensor_add(sq[:, 0:3, :], sq[:, 0:3, :], sq[:, 4:7, :])
                nc.vector.tensor_add(sq[:, 0:2, :], sq[:, 0:2, :], sq[:, 2:4, :])
                nc.vector.tensor_add(sq[:, 0:1, :], sq[:, 0:1, :], sq[:, 1:2, :])
                # bc carries the XSCALE factor the normalized path had, so the
                # drains' scale*DESCALE stays uniform across both paths
                bch = make_bc(ktn=KTH, skip_sq=True, to_sbuf=True)

                if mode == "pe":
                    w1c = hp.tile([128, KTH, 512], fp16, tag="w1c")
                    nc.vector.memset(w1c[:], 0.001)
                for g in range(NG):
                    if mode == "pe":
                        w1 = w1c
                    elif g < 3:
                        w1 = w1_pre[g]
                    else:
                        w1 = w1p.tile([128, KTH, 512], fp16, tag="w1")
                        nc.sync.dma_start(w1[:], dr[f"wp1_g{g}"][:])
                        res_dma(2)
                        res_dma(3)
                    for mloc in range(NG):
                        m = g * NG + mloc
                        zp = pz.tile([128, 512], fp32, tag="z")
                        zv = zp[:, :R]
                        for kt in range(KTH):
                            nc.tensor.matmul(
                                zv, w1[:, kt, mloc * 128:(mloc + 1) * 128],
                                dh[:, kt, :], start=(kt == 0), stop=(kt == KTH - 1))
                        t_ = tp.tile([128, R], fp32, tag="tmp")
                        nc.vector.tensor_mul(t_[:], zv, bch[:, :R])
                        nc.scalar.activation(hp07[:, m, :], t_[:], AF.Relu,
                                             bias=bt[:, m, 0:1],
                                             scale=0.7 * DESCALE)
                        # t0: s1 = hp07 + c1 is pure scratch (t1's Ws1-hoist
                        # overwrites s1), so fuse the add and the fp16 cast:
                        # d1@t0 = fp16(hp07 + c1) UN-normalized, one vector op
                        # (consumers Wp2/Ws1@t0 bc-multiply in their drains)
                        nc.vector.tensor_scalar_add(d1[:, m, :], hp07[:, m, :],
                                                    bt[:, m, 8:9])

            # ---------------- main loop: 16 big matmuls --------------------
            with tc.tile_pool(name="wstream", bufs=4) as wp:
                if mode == "pe":
                    wt0 = pp.tile([128, KT, 512], fp16, tag="wt0")
                    nc.vector.memset(wt0[:], 0.001)

                def big_mm(widx, dsrc, drain, after_g1=None):
                    # after_g1: hoisted-norm hook fired at the END of group 1.
                    # The PE is in-order, so the norm's rowsum/bc matmuls
                    # block the next group's matmuls until the norm's vector
                    # square/tree chain resolves — firing one group later
                    # gives that chain a full group of matmul cover.
                    resident = {}
                    if mode != "pe" and widx == WIDX["Wp2"]:
                        resident = dict(enumerate(wp2res))
                    for g in range(NG):
                        if g in resident:
                            wt = resident[g]
                        elif mode == "pe":
                            wt = wt0
                        else:
                            wt = wp.tile([128, KT, 512], fp16, tag="w")
                            nc.sync.dma_start(wt[:], dr[f"w{widx}_g{g}"][:])
                        for mloc in range(NG):
                            m = g * NG + mloc
                            zp = pz.tile([128, 512], fp32, tag="z")
                            zv = zp[:, :R]
                            for kt in range(KT):
                                nc.tensor.matmul(
                                    zv, wt[:, kt, mloc * 128:(mloc + 1) * 128],
                                    dsrc[:, kt, :],
                                    start=(kt == 0), stop=(kt == KT - 1))
                            drain(m, zv)
                        if g == 1 and after_g1 is not None:
                            after_g1()

                def d_first(nbuf, col, scale):
                    def f(m, zv):
                        nc.scalar.activation(nbuf[:, m, :], zv, AF.Relu,
                                             bias=bt[:, m, col:col + 1],
                                             scale=scale * DESCALE)
                    return f

                def d_add(nbuf, col, scale, extra=None):
                    def f(m, zv):
                        t = tp.tile([128, R], fp32, tag="tmp")
                        nc.scalar.activation(t[:], zv, AF.Relu,
                                             bias=bt[:, m, col:col + 1],
                                             scale=scale * DESCALE)
                        nc.vector.tensor_add(nbuf[:, m, :], nbuf[:, m, :], t[:])
                        if extra == "hp07":
                            nc.vector.tensor_add(
                                nbuf[:, m, :], nbuf[:, m, :], hp07[:, m, :])
                        elif extra is not None:  # const bias column index
                            nc.vector.tensor_scalar_add(
                                nbuf[:, m, :], nbuf[:, m, :], bt[:, m, extra:extra + 1])
                    return f

                def d_c(nbuf, col, scale, cc):
                    def f(m, zv):
                        nc.scalar.activation(nbuf[:, m, :], zv, AF.Relu,
                                             bias=bt[:, m, col:col + 1],
                                             scale=scale * DESCALE)
                        nc.vector.tensor_scalar_add(
                            nbuf[:, m, :], nbuf[:, m, :], bt[:, m, cc:cc + 1])
                    return f

                def bc_mul(drain_fn, bc):
                    # for mms consuming an UN-normalized fp16 state: apply the
                    # row-broadcast XSCALE/(||row||+EPS) to the PSUM result
                    # first; the inner drain's scale*DESCALE then cancels the
                    # factors exactly as on the normalized path
                    def f(m, zv):
                        t_ = tp.tile([128, R], fp32, tag="tmp")
                        nc.vector.tensor_mul(t_[:], zv, bc[:, :R])
                        drain_fn(m, t_[:])
                    return f

                # ---- t0 ----  (d1 = fp16(hp07 + c1) built in the Wp1 drain;
                # sumsq as slab ops here from d1 — its fp16 rounding shifts
                # ||row|| by ~2^-11, the same differential-noise class the
                # precision study cleared. d1@t0 is UN-normalized: Wp2/Ws1
                # start immediately and apply bc1 in their drains, so the inv
                # chain never stalls the PE)
                nc.scalar.activation(sq[:], d1[:], AF.Square)
                nc.vector.tensor_add(sq[:, 0:8, :], sq[:, 0:8, :], sq[:, 8:16, :])
                nc.vector.tensor_add(sq[:, 0:4, :], sq[:, 0:4, :], sq[:, 4:8, :])
                nc.vector.tensor_add(sq[:, 0:2, :], sq[:, 0:2, :], sq[:, 2:4, :])
                nc.vector.tensor_add(sq[:, 0:1, :], sq[:, 0:1, :], sq[:, 1:2, :])
                if mode == "pe":
                    bc1 = make_bc(skip_sq=True, to_sbuf=True)
                    big_mm(WIDX["Wp2"], d1, bc_mul(d_c(s2, 3, 0.7, 9), bc1))
                else:
                    # Wp2@t0 with DEFERRED group-0 drains: the PE is in-order,
                    # so emitting its first 4 chains (resident weights, d1
                    # ready) ahead of bc1's rowsum/bc matmuls covers the s1
                    # square/tree vector chain those wait on
                    zvs = []
                    for mloc in range(NG):
                        zp = pz.tile([128, 512], fp32, tag="z")
                        zv = zp[:, :R]
                        for kt in range(KT):
                            nc.tensor.matmul(
                                zv,
                                wp2res[0][:, kt, mloc * 128:(mloc + 1) * 128],
                                d1[:, kt, :],
                                start=(kt == 0), stop=(kt == KT - 1))
                        zvs.append(zv)
                    bc1 = make_bc(skip_sq=True, to_sbuf=True)
                    dr0 = bc_mul(d_c(s2, 3, 0.7, 9), bc1)
                    for mloc in range(NG):
                        dr0(mloc, zvs[mloc])
                    for g in range(1, NG):
                        for mloc in range(NG):
                            m = g * NG + mloc
                            zp = pz.tile([128, 512], fp32, tag="z")
                            zv = zp[:, :R]
                            for kt in range(KT):
                                nc.tensor.matmul(
                                    zv,
                                    wp2res[g][:, kt,
                                              mloc * 128:(mloc + 1) * 128],
                                    d1[:, kt, :],
                                    start=(kt == 0), stop=(kt == KT - 1))
                            dr0(m, zv)
                # t1's Ws1 term hoisted here: only needs d1, covers norm(s2)
                # (0.3*s-part first, then 0.7*q-part added: commutative, bit-exact)
                # norm(s2) runs inside Ws1's group 0 so Wp3 never stalls on d2
                big_mm(WIDX["Ws1"], d1, bc_mul(d_first(s1, 2, 0.3), bc1),
                       after_g1=lambda: norm(s2, d2))
                big_mm(WIDX["Wp3"], d2, d_c(s3, 6, 0.7, 10))

                # ---- t1, t2 ----
                # d1/d2 already hold norm(s1)/norm(s2) at each iteration entry
                def d_add_g(nbuf, col, scale, extra):
                    # d_add + per-chunk square/tree for goodness (same pairs)
                    base = d_add(nbuf, col, scale, extra=extra)

                    def f(m, zv):
                        base(m, zv)
                        nc.scalar.activation(sq[:, m, :], nbuf[:, m, :], AF.Square)
                        if m >= 8:
                            nc.vector.tensor_add(
                                sq[:, m - 8, :], sq[:, m - 8, :], sq[:, m, :])
                    return f

                for _t in (1, 2):
                    # norm(s3,d3) issued after Wq1's first group: its PE ops
                    # then never stall (square/tree overlap group 0 matmuls)
                    big_mm(WIDX["Wq1"], d2, d_add(s1, 1, 0.7, extra="hp07"),
                           after_g1=lambda: norm(s3, d3))
                    # norm(s1) hoisted into Wq2's group 0 (s1 is final after
                    # Wq1) so Wp2 never stalls on d1; at t2 the norm's own
                    # row-sumsq IS the s1 goodness term — capture it free
                    gc1 = "init" if _t == 2 else None
                    big_mm(WIDX["Wq2"], d3, d_first(s2, 4, 0.7),
                           after_g1=lambda gc=gc1: norm(s1, d1, g_capture=gc))
                    big_mm(WIDX["Ws2"], d2, d_add(s2, 5, 0.3))
                    big_mm(WIDX["Wp2"], d1, d_add(s2, 3, 0.7))
                    # norm(s2) hoisted likewise (s2 final after Wp2); at t2 it
                    # captures the s2 goodness term
                    gc2 = "add" if _t == 2 else None
                    if _t == 1:
                        big_mm(WIDX["Ws1"], d1, d_first(s1, 2, 0.3),  # t2 hoist
                               after_g1=lambda: norm(s2, d2))
                        big_mm(WIDX["Ws3"], d3, d_first(s3, 7, 0.3))
                        big_mm(WIDX["Wp3"], d2, d_add(s3, 6, 0.7, extra=11))
                    else:
                        big_mm(WIDX["Ws3"], d3, d_first(s3, 7, 0.3),
                               after_g1=lambda gc=gc2: norm(s2, d2, g_capture=gc))
                        big_mm(WIDX["Wp3"], d2, d_add_g(s3, 6, 0.7, extra=11))

                # ---- goodness tail (s3): finish tree, rowsum, fold into grow
                nc.vector.tensor_add(sq[:, 0:4, :], sq[:, 0:4, :], sq[:, 4:8, :])
                nc.vector.tensor_add(sq[:, 0:2, :], sq[:, 0:2, :], sq[:, 2:4, :])
                nc.vector.tensor_add(sq[:, 0:1, :], sq[:, 0:1, :], sq[:, 1:2, :])

                zg = pn.tile([128, 512], fp32, tag="ss")
                nc.tensor.matmul(zg[0:1, :R], ones_col[:], sq[:, 0, :],
                                 start=True, stop=True)
                nc.vector.tensor_add(grow[:], grow[:], zg[0:1, :R])
                for s in range(SPC):
                    nc.vector.max_with_indices(
                        mxrow[0:1, s * 8:(s + 1) * 8],
                        ixrow[0:1, s * 8:(s + 1) * 8],
                        grow[0:1, s * L:(s + 1) * L])
                nc.vector.tensor_copy(outf[:], ixrow[0:1, 0:8 * SPC:8])
                nc.sync.dma_start(out_d[:], outf[:])

    nc.finalize()
    return nc


def make_in_maps(inputs):
    sh = prep_shared(inputs)
    return [{**sh, **prep_core(inputs, c)} for c in range(NC_)]


_NC_CACHE = None


def kernel(**inputs):
    from concourse.bass_utils import run_bass_kernel_spmd
    global _NC_CACHE
    if _NC_CACHE is None:
        _NC_CACHE = build_program()
    in_maps = make_in_maps(inputs)
    res = run_bass_kernel_spmd(_NC_CACHE, in_maps, core_ids=list(range(NC_)))
    outs = [np.asarray(res.results[c]["out"]) for c in range(NC_)]
    return np.concatenate(outs, axis=0).astype(np.float32)

